# revision 3
# baseline (speedup 1.0000x reference)
"""Trainium2 Bass kernel for nn_Conv2dBN_fake_int8.

Math: the reference quantizes x and weight to int8 levels, then computes
out[b,l,o] = sum_k lut[qf[b,l,k]+128, qw[o,k]+128] with lut the exact
product table lut[i,j] = (i-128)*(j-128), so the LUT-GEMM is an integer
GEMM == a 3x3 pad-1 conv on the quantized values.  We verify the product
property of the passed lut on the host (cheap) and run the conv on the
TensorEngine in bf16 (all products/partial sums are integers < 2^24, so
fp32 PSUM accumulation is exact).

Weights are quantized/packed on the host (offline weight quant, the
standard int8-deployment contract) directly in lhsT layout, so the device
never touches the weight path.  x is shipped twice (lower plane = x,
upper plane = x shifted one image row, zero tail) so one ACT op + one DVE
op per pixel-half quantizes all 128 partitions; the row-shifted upper
plane lets each of the three (kh=1,kh=2) tap pairs run as a single K=128
matmul.  Per 16-row output chunk: 3 single-tap (kh=0) K=64 matmuls plus
3 pair K=128 matmuls accumulate into one PSUM bank.

Dequant: d1 = acc*s2 + b2 on ACT (per-channel scale/bias), then DVE
round via +/-1.5*2^23 magic, then (mult sa, max lo)(min hi) which equals
the reference's clip-then-scale bit-exactly (fp32 mult is monotone and
the bounds are fp32(+-128*sa)).  Chunk stores are issued from ACT and SP
(the two HWDGE engines) so descriptor generation overlaps.

Sharding: data-parallel over batch B=8 across the 8 NeuronCores (one
image per core); weights/scales replicated.
"""

import numpy as np

# Problem shape (hardcoded; harness runs kernel.py standalone).
B, C, H, W = 8, 64, 32, 32
O, KH, KW = 64, 3, 3
OH, OW = 32, 32
L = OH * OW          # 1024
NT = KH * KW         # 9 taps
K = C * NT           # 576
PADW = W + 2         # 34
PADA = (H + 2) * PADW  # 1156
NCORES = 8
CHUNK = 512          # fp32 free elements per PSUM bank
RPC = CHUNK // OW    # output rows per PSUM chunk (16)
MAGIC = 12582912.0   # 1.5*2^23 -> fp32 round-to-nearest-even via add/sub
H0ROWS = 17          # first quantize half: 17 rows so chunk0's pair taps
H1ROWS = H - H0ROWS  # (which read one row past the chunk) stay inside it
WSB_COLS = 6 * O + 4  # [3 pair blocks | 3 single blocks | s2 | b2] bf16 cols

_nc_cache = {}


def _make_tc_class():
    """TileContext whose kernel-tail drain is split into a chain of
    single-wait Drain instructions: the walrus build used here allows only
    one sync-wait command per instruction, while stock Tile emits one drain
    waiting on every processor at once.  Sequentially waiting on the same
    set of semaphores is synchronization-equivalent."""
    import concourse.tile as tile
    from concourse import mybir
    from concourse.vector_clock import ScopedClock

    class SingleWaitDrainTC(tile.TileContext):
        def _drain_and_barrier(self, tick_clock, wait_clock):
            drain_inst = self.nc.sync.drain()
            wait_clock.add_sem_waits(
                drain_inst.ins, ScopedClock({None: tick_clock.global_clock})
            )
            si = drain_inst.ins.sync_info
            if si is not None and len(si.on_wait) > 1:
                waits = list(si.on_wait)
                updates = list(si.on_update)
                drain_inst.ins.sync_info = mybir.SyncInfo(
                    on_wait=waits[:1], on_update=[]
                )
                for i, w in enumerate(waits[1:]):
                    d = self.nc.sync.drain()
                    last = i == len(waits) - 2
                    d.ins.sync_info = mybir.SyncInfo(
                        on_wait=[w], on_update=updates if last else []
                    )
            self.nc.all_engine_barrier()
            assert self.sems is not None
            popped = self.nc._tile_sem_poison_stack.pop()
            assert popped is self._sem_poison
            self.nc.clear_and_free_semaphores(list(self.sems.allocated().values()))
            self.nc.all_engine_barrier()

    return SingleWaitDrainTC


def _build(sf: float, sa: float, clip_x: bool):
    import concourse.bass as bass
    import concourse.tile as tile
    from concourse import mybir

    dt = mybir.dt
    alu = mybir.AluOpType
    act = mybir.ActivationFunctionType

    nc = bass.Bass(
        "TRN2",
        debug=False,
        enable_asserts=False,
        target_bir_lowering=False,
        num_devices=NCORES,
    )

    x_d = nc.dram_tensor("x", [2 * C, L], dt.float32, kind="ExternalInput").ap()
    wsb_d = nc.dram_tensor(
        "wsb", [2 * C, WSB_COLS], dt.bfloat16, kind="ExternalInput"
    ).ap()
    out_d = nc.dram_tensor("out", [O, L], dt.float32, kind="ExternalOutput").ap()

    inv_sf = float(np.float32(1.0) / np.float32(sf))
    sa_f = float(np.float32(sa))
    clip_lo = float(np.float32(-128.0) * np.float32(sa))
    clip_hi = float(np.float32(127.0) * np.float32(sa))
    PX0 = H0ROWS * OW  # 544 pixels in the first half

    with _make_tc_class()(nc) as tc:
        with (
            tc.tile_pool(name="per", bufs=1) as per,
            tc.tile_pool(name="dq", bufs=2) as dq,
            tc.tile_pool(name="ps_acc", bufs=2, space="PSUM") as ps_acc,
        ):
            # ---------------- loads ----------------
            # x (both planes) in two pixel-halves on the ACT HWDGE ring;
            # weights+scales on the SP ring - all descriptor generation
            # happens immediately after the boot barrier.
            xs = per.tile([2 * C, L], dt.float32)
            nc.scalar.dma_start(out=xs[:, 0:PX0], in_=x_d[:, 0:PX0])
            nc.scalar.dma_start(out=xs[:, PX0:L], in_=x_d[:, PX0:L])

            wsb = per.tile([2 * C, WSB_COLS], dt.bfloat16)
            nc.sync.dma_start(out=wsb, in_=wsb_d)
            wT = wsb[:, 0 : 6 * O]
            s2_sb = wsb[0:O, 6 * O : 6 * O + 2].bitcast(dt.float32)
            b2_sb = wsb[0:O, 6 * O + 2 : 6 * O + 4].bitcast(dt.float32)

            # early ACT touch of wsb so the dequant Activations only need a
            # single (PE) wait later - covers the wsb DMA queue on ACT.
            act_cover = per.tile([O, 1], dt.float32)

            # ------- zero the pad cells the matmuls read -------
            qxa = per.tile([2 * C, PADA], dt.bfloat16)
            qa3 = qxa.rearrange("c (r col) -> c r col", col=PADW)
            # pad row 0 of the lower plane (kh=0 taps, chunk 0)
            nc.vector.memset(qxa[0:C, 0:PADW], 0.0)
            # right pad col of row r + left pad col of row r+1, rows 0..32,
            # on BOTH planes (one strided memset across all 128 partitions)
            side_pads = bass.AP(
                tensor=qxa.tensor, offset=qxa.offset + W + 1,
                ap=[qxa.ap[0], [PADW, H + 1], [1, 2]],
            )
            nc.vector.memset(side_pads, 0.0)

            # ------- quantize x -> bf16 into both planes at once -------
            # qf = round_half_even(x * (1/sf)) [clip optional: the host
            # checked the input range].  Upper plane data is pre-shifted on
            # the host, so one [128,*] write fills lower+upper.
            t1 = per.tile([2 * C, L], dt.float32)
            halves = [(0, 0, H0ROWS), (PX0, H0ROWS, H1ROWS)]
            for p0, r0h, nh in halves:
                px = slice(p0, p0 + nh * OW)
                nc.scalar.activation(
                    out=t1[:, px], in_=xs[:, px], func=act.Copy,
                    scale=inv_sf, bias=MAGIC,
                )
                tgt = qa3[:, 1 + r0h : 1 + r0h + nh, 1 : W + 1]
                src = t1[:, px].rearrange("c (r col) -> c r col", col=W)
                if clip_x:
                    tq = per.tile([2 * C, nh * OW], dt.float32, tag="tq")
                    nc.vector.tensor_scalar(
                        out=tq, in0=t1[:, px], scalar1=MAGIC, scalar2=-128.0,
                        op0=alu.subtract, op1=alu.max,
                    )
                    nc.vector.tensor_scalar(
                        out=tgt,
                        in0=tq.rearrange("c (r col) -> c r col", col=W),
                        scalar1=127.0, scalar2=None, op0=alu.min,
                    )
                else:
                    nc.vector.tensor_scalar(
                        out=tgt, in0=src, scalar1=MAGIC, scalar2=None,
                        op0=alu.subtract,
                    )

            nc.scalar.mul(act_cover, s2_sb, 1.0)

            # ------- conv: 3 single + 3 pair matmuls per 16-row chunk -------
            acc0 = ps_acc.tile([O, CHUNK], dt.float32, tag="acc0")
            acc1 = ps_acc.tile([O, CHUNK], dt.float32, tag="acc1")
            accs = [acc0, acc1]
            for n in range(L // CHUNK):
                r0 = n * RPC
                acc = accs[n]
                for kw in range(KW):  # kh=0 taps: K=64 from the lower plane
                    nc.tensor.matmul(
                        acc, wT[0:C, (3 + kw) * O : (4 + kw) * O],
                        qa3[0:C, r0 : r0 + RPC, kw : kw + OW],
                        start=(kw == 0), stop=False,
                    )
                for kw in range(KW):  # (kh=1, kh=2) pairs: K=128
                    nc.tensor.matmul(
                        acc, wT[:, kw * O : (kw + 1) * O],
                        qa3[:, 1 + r0 : 1 + r0 + RPC, kw : kw + OW],
                        start=False, stop=(kw == KW - 1),
                    )

            # ------- dequant + fake-quant + store (per chunk) -------
            # ref: y = acc*sf*sw + bias; y = round(y/sa); clip; y*sa
            store_engines = [nc.scalar, nc.sync]
            for n in range(L // CHUNK):
                sl = slice(n * CHUNK, (n + 1) * CHUNK)
                d1 = dq.tile([O, CHUNK], dt.float32, tag="d1")
                nc.scalar.activation(
                    out=d1, in_=accs[n], func=act.Identity,
                    scale=s2_sb, bias=b2_sb,
                )
                d2 = dq.tile([O, CHUNK], dt.float32, tag="d2")
                nc.vector.tensor_scalar(
                    out=d2, in0=d1, scalar1=MAGIC, scalar2=MAGIC,
                    op0=alu.add, op1=alu.subtract,
                )
                d3 = dq.tile([O, CHUNK], dt.float32, tag="d3")
                nc.vector.tensor_scalar(
                    out=d3, in0=d2, scalar1=sa_f, scalar2=clip_lo,
                    op0=alu.mult, op1=alu.max,
                )
                d4 = dq.tile([O, CHUNK], dt.float32, tag="d4")
                nc.vector.tensor_scalar(
                    out=d4, in0=d3, scalar1=clip_hi, scalar2=None, op0=alu.min,
                )
                store_engines[n % 2].dma_start(out=out_d[:, sl], in_=d4)

    return nc


def _get_nc(scale_feature, scale_activation, clip_x):
    sf = float(np.float32(scale_feature))
    sa = float(np.float32(scale_activation))
    key = (sf, sa, bool(clip_x))
    if key not in _nc_cache:
        _nc_cache[key] = _build(sf, sa, clip_x)
    return _nc_cache[key]


def _make_in_maps(x, weight, scale_weight, bias, scale_feature, scale_activation):
    import ml_dtypes

    sf = np.float32(scale_feature)
    sa = np.float32(scale_activation)
    sw = scale_weight.reshape(O).astype(np.float32)
    b = bias.reshape(O).astype(np.float32)
    s2 = (sf * sw) / sa                      # fp32 per-channel dequant scale
    b2 = b / sa                              # fp32 bias in activation-steps

    # Host weight quantization (offline int8 weight quant) packed straight
    # into lhsT block layout: blocks 0-2 = (kh=1,kh=2) pairs per kw,
    # blocks 3-5 = kh=0 singles per kw (upper 64 rows zero).
    qw = np.clip(
        np.round(weight.reshape(O, C, KH, KW) / sw[:, None, None, None]),
        -128.0, 127.0,
    ).astype(np.float32)
    wT = np.zeros((2 * C, WSB_COLS), dtype=ml_dtypes.bfloat16)
    for kw in range(KW):
        wT[0:C, kw * O : (kw + 1) * O] = qw[:, :, 1, kw].T
        wT[C : 2 * C, kw * O : (kw + 1) * O] = qw[:, :, 2, kw].T
        wT[0:C, (3 + kw) * O : (4 + kw) * O] = qw[:, :, 0, kw].T
    wsb16 = wT.view(np.uint16)
    wsb16[0:O, 6 * O : 6 * O + 2] = s2.astype("<f4").view("<u2").reshape(O, 2)
    wsb16[0:O, 6 * O + 2 : 6 * O + 4] = b2.astype("<f4").view("<u2").reshape(O, 2)

    xr = x.reshape(B, C, L).astype(np.float32)
    zeros = np.zeros((C, OW), np.float32)
    maps = []
    for bb in range(B):
        xlo = xr[bb]
        xup = np.concatenate([xlo[:, OW:], zeros], axis=1)
        maps.append({
            "x": np.ascontiguousarray(np.concatenate([xlo, xup], axis=0)),
            "wsb": np.ascontiguousarray(wsb16.view(ml_dtypes.bfloat16)),
        })
    return maps


def _kernel_device(x, weight, scale_feature, scale_weight, scale_activation, bias):
    from concourse import bass_utils

    sf = np.float32(scale_feature)
    v = x.astype(np.float32) * (np.float32(1.0) / sf)
    clip_x = not (float(v.min()) >= -128.5 and float(v.max()) < 127.5)
    nc = _get_nc(scale_feature, scale_activation, clip_x)
    in_maps = _make_in_maps(
        x, weight, scale_weight, bias, scale_feature, scale_activation
    )
    res = bass_utils.run_bass_kernel_spmd(nc, in_maps, core_ids=list(range(NCORES)))
    return np.stack([r["out"].reshape(O, OH, OW) for r in res.results]).astype(
        np.float32
    )


def _kernel_numpy_lut(x, weight, lut, sf, sw, sa, bias):
    """Honest LUT-GEMM fallback (only if lut is not the product table)."""
    qf = np.clip(np.round(x / np.float32(sf)), -128.0, 127.0)
    qw = np.clip(np.round(weight / sw[:, None, None, None]), -128.0, 127.0)
    idx_w = qw.reshape(O, K).astype(np.int64) + 128
    qfp = np.pad(qf, ((0, 0), (0, 0), (1, 1), (1, 1)))
    acc = np.zeros((B, L, O), np.int64)
    for t in range(NT):
        kh, kw = divmod(t, KW)
        win = qfp[:, :, kh : kh + OH, kw : kw + OW].reshape(B, C, L)
        idx_f = win.astype(np.int64) + 128  # [B, C, L]
        for c in range(C):
            acc += lut[idx_f[:, c, :, None], idx_w[None, None, :, c * NT + t]]
    out = acc.astype(np.float32).transpose(0, 2, 1).reshape(B, O, OH, OW)
    out = out * np.float32(sf) * sw[None, :, None, None]
    out = out + bias[None, :, None, None]
    out = np.round(out / np.float32(sa))
    out = np.clip(out, -128.0, 127.0)
    return (out * np.float32(sa)).astype(np.float32)


def kernel(x, weight, lut, scale_feature, scale_weight, scale_activation, bias):
    x = np.asarray(x, dtype=np.float32)
    weight = np.asarray(weight, dtype=np.float32)
    lut = np.asarray(lut)
    scale_weight = np.asarray(scale_weight, dtype=np.float32)
    bias = np.asarray(bias, dtype=np.float32)

    i = np.arange(256, dtype=np.int64) - 128
    product = i[:, None] * i[None, :]
    if not np.array_equal(np.asarray(lut, dtype=np.int64), product):
        return _kernel_numpy_lut(
            x, weight, np.asarray(lut, dtype=np.int64),
            float(np.float32(scale_feature)), scale_weight,
            float(np.float32(scale_activation)), bias,
        )

    return _kernel_device(
        x, weight, scale_feature, scale_weight, scale_activation, bias
    )
